# revision 26
# baseline (speedup 1.0000x reference)
"""Trainium2 Bass kernel for the CPC loss problem (nn_CPC_85117661872355).

Strategy (data-parallel over batch B across 8 cores):
  - Each core handles 8 of the 64 batch elements: 1120 prediction rows.
  - Phase 1 computes pred TRANSPOSED: predT[e, r] = sum_d Wk[s][e,d] ctx[r,d]
    + b[e], as a single-pass fp16 matmul (Wk blocks stationary, ctx^T rows
    moving, fp32 PSUM accumulate).  E lands on partitions, so the ragged
    per-s row groups go on the free axis and no repack is needed.
  - All 17 logits per row (1 positive + 16 negatives) are dot products
    pred_row . enc_flat[idx], contracted over E.  Target vectors are fetched
    with SWDGE dma_gather(transpose=True) from an fp16 copy of the encoding
    table, which lands them E-on-partitions: gtT[e_sub, e_blk, j].  Each
    128-dot group is then a 10-matmul PE accumulation
    out[r, j] = sum_e predT[e, r] gtT[e, j] whose DIAGONAL holds the dots;
    a fused DVE scalar_tensor_tensor against a host-supplied identity
    extracts diag+accumulates in one [128,128] op.  This moves the 25M
    multiply-adds of the dot products from DVE (1 elem/cycle) to the PE.
  - Gathering the positive through the same path keeps bitwise ties when a
    negative index collides with the positive (jnp.argmax first-index
    tie-break).
  - Dots tiles for all 9 supergroups are kept; softmax-CE runs once at the
    end (one Exp table load, one Ln), accumulating loss/correct per
    partition; a ones-matmul reduces to [1,2] per core; host sums cores.
"""

import functools

import numpy as np

import concourse.bass as bass
import concourse.mybir as mybir
import concourse.tile as tile
from concourse import bacc
from concourse.bass_utils import run_bass_kernel_spmd

F32 = mybir.dt.float32
FP16 = mybir.dt.float16

B, G, D = 64, 7, 1280
S, NEG = 5, 16
NCORES = 8
BSH = B // NCORES  # 8
NS = [BSH * (6 - s) * G for s in range(S)]  # [336, 280, 224, 168, 112]
SOFF = [0]
for n in NS:
    SOFF.append(SOFF[-1] + n)
NR = SOFF[-1]  # 1120 rows per core
NSG = 9  # supergroups of 128 rows
NDOT = 17  # 1 positive + 16 negatives
GCHUNKS = [(0, 6), (6, 6), (12, 5)]  # gather (goff, width) per supergroup
IDX_PER_SG = NDOT * 128  # 2176
IDX_TOT = NSG * IDX_PER_SG  # 19584
N_PREDS = B * G * 20  # 8960

# Results of the last device run (for test harness introspection)
LAST_RUN = {}


@functools.lru_cache(maxsize=1)
def build_nc() -> bass.Bass:
    nc = bacc.Bacc(
        "TRN2",
        target_bir_lowering=False,
        debug=False,
        num_devices=NCORES,
        num_swdge_queues=1,
    )
    # ctxT: [d, r] with d split [128 d_sub, 10 d_blk]
    ctxT = nc.declare_dram_parameter("ctxT", [D, NR], FP16, isOutput=False)
    # wk5: [128 d_in_sub, S, 10 d_out_blk(e), 10 d_in_blk, 128 e_sub]
    # element [di, s, eo, do, ei] = Wk_w[s, eo*128+ei, do*128+di]
    wk5 = nc.declare_dram_parameter("wk5", [128, S, 10, 10, 128], FP16,
                                    isOutput=False)
    wkb = nc.declare_dram_parameter("wkb", [1, S, 10, 128], FP16,
                                    isOutput=False)
    ench = nc.declare_dram_parameter("ench", [B * G * G, D], FP16,
                                     isOutput=False)
    ident = nc.declare_dram_parameter("ident", [128, 128], FP16,
                                      isOutput=False)
    idx = nc.declare_dram_parameter(
        "idx", [128, IDX_TOT // 16], mybir.dt.int16, isOutput=False
    )
    out = nc.declare_dram_parameter("out", [1, 2], F32, isOutput=True)

    Alu = mybir.AluOpType
    Act = mybir.ActivationFunctionType
    Ax = mybir.AxisListType

    # supergroups fully covered after each s finishes phase 1
    SG_AFTER_S = [[0, 1], [2, 3], [4, 5], [6], [7, 8]]
    SG_M = [128] * 8 + [96]  # valid rows per supergroup

    with tile.TileContext(nc) as tc:
        with (
            tc.tile_pool(name="const", bufs=1) as constp,
            tc.tile_pool(name="wk", bufs=2) as wkp,
            tc.tile_pool(name="gath", bufs=3) as gathp,
            tc.tile_pool(name="gath1", bufs=2) as gath1p,
            tc.tile_pool(name="dots", bufs=NSG) as dotsp,
            tc.tile_pool(name="small", bufs=4) as smallp,
            tc.tile_pool(name="acc", bufs=1) as accp,
            tc.tile_pool(name="psA", bufs=2, space="PSUM") as psAp,
            tc.tile_pool(name="psD", bufs=3, space="PSUM") as psDp,
            tc.tile_pool(name="psF", bufs=1, space="PSUM") as psFp,
        ):
            # ---- constants ----
            idx_sb = constp.tile([128, IDX_TOT // 16], mybir.dt.int16,
                                 tag="idx")
            nc.sync.dma_start(idx_sb[:, :], idx[:, :])
            ident_sb = constp.tile([128, 128], FP16, tag="ident")
            nc.sync.dma_start(ident_sb[:, :], ident[:, :])
            ones_sb = constp.tile([128, 1], F32, tag="ones")
            nc.vector.memset(ones_sb[:, :], 1.0)
            ones16 = constp.tile([1, 512], FP16, tag="ones16")
            nc.vector.memset(ones16[:, :], 1.0)
            wkb_sb = constp.tile([1, S, 10, 128], FP16, tag="wkb")
            nc.sync.dma_start(wkb_sb[:, :, :, :], wkb[:, :, :, :])

            # resident fp16 ctx^T: [128 d_sub, 10 d_blk, NR rows]
            ctx_sb = constp.tile([128, 10, NR], FP16, tag="ctx")
            ctx_r = ctxT[:, :].rearrange("(do di) r -> di do r", di=128)
            nc.sync.dma_start(ctx_sb[:, :, 0:560], ctx_r[:, :, 0:560])
            nc.sync.dma_start(ctx_sb[:, :, 560:NR], ctx_r[:, :, 560:NR])

            # predT resident: [128 e_sub, 10 e_blk, NR rows] fp16
            predT = constp.tile([128, 10, NR], FP16, tag="predT")

            # CE stat accumulators
            ss9 = accp.tile([128, NSG], F32, tag="ss9")
            pm9 = accp.tile([128, NSG], F32, tag="pm9")
            lc18 = accp.tile([128, 2 * NSG], F32, tag="lc18")
            scr128 = accp.tile([128, 128], FP16, tag="scr128")

            # ---- gathers (transposed): issued with small lookahead ----
            ench_ap = ench[:, :]
            gt_tiles = {}
            gcursor = [0]

            def emit_gathers(upto):
                while gcursor[0] < min(upto, NSG * len(GCHUNKS)):
                    k = gcursor[0]
                    sg, ci = divmod(k, len(GCHUNKS))
                    goff, w = GCHUNKS[ci]
                    nidx = w * 128
                    pool = gathp if w == 6 else gath1p
                    gt = pool.tile([128, 10, nidx], FP16, tag=f"gt{w}")
                    pos0 = sg * IDX_PER_SG + goff * 128
                    nc.gpsimd.dma_gather(
                        gt[:, :, :],
                        ench_ap,
                        idx_sb[:, pos0 // 16 : (pos0 + nidx) // 16],
                        nidx,
                        nidx,
                        D,
                        transpose=True,
                        queue_num=0,
                    )
                    gt_tiles[(sg, ci)] = gt
                    gcursor[0] += 1

            emit_gathers(3)

            dots_tiles = [
                dotsp.tile([128, NDOT], F32, tag="dots", name=f"dots{i}")
                for i in range(NSG)
            ]
            # rows 96.. of supergroup 8 never get extracts; keep them finite
            nc.vector.memset(dots_tiles[8][96:128, :], 0.0)

            def emit_phase2(sg):
                M = SG_M[sg]
                r0 = sg * 128
                dots_t = dots_tiles[sg]
                for ci, (goff, w) in enumerate(GCHUNKS):
                    gt = gt_tiles.pop((sg, ci))
                    for j in range(w):
                        g = goff + j
                        dps = psDp.tile([128, 512], F32, tag="dps")
                        for dblk in range(10):
                            nc.tensor.matmul(
                                dps[:M, 0:128],
                                lhsT=predT[:, dblk, r0 : r0 + M],
                                rhs=gt[:, dblk, j * 128 : (j + 1) * 128],
                                start=(dblk == 0),
                                stop=(dblk == 9),
                            )
                        # diag extract: accum((psum * 1.0) * I) -> dots col
                        nc.vector.scalar_tensor_tensor(
                            scr128[:M, :],
                            dps[:M, 0:128],
                            1.0,
                            ident_sb[:M, :],
                            op0=Alu.mult,
                            op1=Alu.mult,
                            accum_out=dots_t[:M, g : g + 1],
                        )
                # per-sg CE stats inline (keeps the tail short)
                negm = smallp.tile([128, 1], F32, tag="negm")
                nc.vector.tensor_reduce(
                    negm[:, :], dots_t[:, :], Ax.X, Alu.max, negate=True
                )
                e_t = smallp.tile([128, NDOT], F32, tag="et")
                nc.scalar.activation(
                    e_t[:, :],
                    dots_t[:, :],
                    Act.Exp,
                    bias=negm[:, 0:1],
                    scale=1.0,
                    accum_out=ss9[:, sg : sg + 1],
                )
                nc.vector.tensor_tensor(
                    pm9[:, sg : sg + 1], dots_t[:, 0:1], negm[:, :], Alu.add
                )
                maxneg = smallp.tile([128, 1], F32, tag="maxneg")
                nc.vector.tensor_reduce(
                    maxneg[:, :], dots_t[:, 1:NDOT], Ax.X, Alu.max
                )
                nc.vector.tensor_tensor(
                    lc18[:, NSG + sg : NSG + sg + 1],
                    dots_t[:, 0:1],
                    maxneg[:, :],
                    Alu.is_ge,
                )
                emit_gathers(len(GCHUNKS) * (sg + 1) + 3)

            # ---- phase 1: predT = Wk[s] @ ctx^T + b (single-pass fp16) ----
            # 128-row tiles so each supergroup's predT completes early and
            # unblocks its diag matmuls (which recycle gather buffers).
            SG_AFTER_RT = {(0, 0): [0], (0, 1): [1], (1, 0): [2], (1, 1): [3],
                           (2, 0): [4], (2, 1): [5], (3, 0): [6],
                           (4, 0): [7, 8]}
            for s in range(S):
                n = NS[s]
                o = SOFF[s]
                wk_t = wkp.tile([128, 10, 10, 128], FP16, tag="wk")
                for dch in range(2):
                    nc.sync.dma_start(
                        wk_t[:, 5 * dch : 5 * dch + 5, :, :],
                        wk5[:, s, 5 * dch : 5 * dch + 5, :, :],
                    )
                for rt in range((n + 127) // 128):
                    r0 = 128 * rt
                    nr = min(128, n - r0)
                    for eblk in range(10):
                        ps = psAp.tile([128, 512], F32, tag="ps")
                        for dblk in range(10):
                            nc.tensor.matmul(
                                ps[:, 0:nr],
                                lhsT=wk_t[:, eblk, dblk, :],
                                rhs=ctx_sb[:, dblk, o + r0 : o + r0 + nr],
                                start=(dblk == 0),
                                stop=False,
                            )
                        # bias: predT[e, r] += b[e] * 1
                        nc.tensor.matmul(
                            ps[:, 0:nr],
                            lhsT=wkb_sb[0:1, s, eblk, :],
                            rhs=ones16[0:1, 0:nr],
                            start=False,
                            stop=True,
                        )
                        nc.scalar.copy(
                            predT[:, eblk, o + r0 : o + r0 + nr], ps[:, 0:nr]
                        )
                    for sg in SG_AFTER_RT.get((s, rt), []):
                        emit_phase2(sg)

            # ---- CE finale: loss_r = ln(ss) - (pos - m) ----
            ln9 = smallp.tile([128, NSG], F32, tag="ln9")
            nc.scalar.activation(ln9[:, :], ss9[:, :], Act.Ln)
            nc.vector.tensor_tensor(
                lc18[:, 0:NSG], ln9[:, :], pm9[:, :], Alu.subtract
            )
            # zero the 32 invalid rows of supergroup 8
            nc.vector.memset(lc18[96:128, 8:9], 0.0)
            nc.vector.memset(lc18[96:128, NSG + 8 : NSG + 9], 0.0)

            # ---- final partition reduce: [128,18] -> [1,18] -> [1,2] ----
            psf = psFp.tile([1, 2 * NSG], F32, tag="psf")
            nc.tensor.matmul(
                psf[:, :], lhsT=ones_sb[:, 0:1], rhs=lc18[:, :], start=True,
                stop=True,
            )
            sum18 = smallp.tile([1, 2 * NSG], F32, tag="sum18")
            nc.vector.tensor_copy(sum18[:, :], psf[:, :])
            outsb = smallp.tile([1, 2], F32, tag="outsb")
            nc.vector.tensor_reduce(
                outsb[:, 0:2],
                sum18[:, :].rearrange("p (a b) -> p a b", a=2),
                Ax.X,
                Alu.add,
            )
            nc.sync.dma_start(out[:, :], outsb[:, :])

    nc.compile()
    return nc


def _row_targets(core: int, neg_idx: np.ndarray) -> np.ndarray:
    """[NR, 17] int array: flat enc index of positive + 16 negatives per row."""
    tg = np.zeros((NR, NDOT), np.int64)
    ri = 0
    for s in range(S):
        rows = 6 - s
        for b in range(BSH):
            bg = core * BSH + b
            for r in range(rows):
                for c7 in range(G):
                    tg[ri, 0] = bg * G * G + (s + 1 + r) * G + c7
                    tg[ri, 1:] = neg_idx[bg, s, r, c7]
                    ri += 1
    assert ri == NR
    return tg


def _build_idx(core: int, neg_idx: np.ndarray) -> np.ndarray:
    """int16 [128, IDX_TOT//16] gather-index tensor in SWDGE wrap layout."""
    tg = _row_targets(core, neg_idx)
    tg_pad = np.zeros((NSG * 128, NDOT), np.int64)
    tg_pad[:NR] = tg
    # list position sg*2176 + g*128 + p  ->  target of (row sg*128+p, dot g)
    lst = tg_pad.reshape(NSG, 128, NDOT).transpose(0, 2, 1).reshape(-1)
    arr = lst.astype(np.int16).reshape(-1, 16).T  # [16, IDX_TOT//16]
    return np.ascontiguousarray(np.tile(arr, (8, 1)))  # [128, ...]


def _prep_in_maps(contexts, encodings, Wk_w, Wk_b, neg_idx):
    contexts = np.ascontiguousarray(np.asarray(contexts, np.float32))
    encodings = np.ascontiguousarray(np.asarray(encodings, np.float32))
    Wk_w = np.ascontiguousarray(np.asarray(Wk_w, np.float32))
    Wk_b = np.ascontiguousarray(np.asarray(Wk_b, np.float32))
    neg_idx = np.asarray(neg_idx)

    ench = np.ascontiguousarray(
        encodings.reshape(B * G * G, D).astype(np.float16)
    )
    # wk5[di, s, eo, do, ei] = Wk_w[s, eo*128+ei, do*128+di]
    wk5 = np.ascontiguousarray(
        Wk_w.reshape(S, 10, 128, 10, 128)
        .transpose(4, 0, 1, 3, 2)
        .astype(np.float16)
    )
    wkb = np.ascontiguousarray(
        Wk_b.reshape(1, S, 10, 128).astype(np.float16)
    )
    identm = np.ascontiguousarray(np.eye(128, dtype=np.float16))

    in_maps = []
    for c in range(NCORES):
        bs = slice(c * BSH, (c + 1) * BSH)
        ctx_rows = np.concatenate(
            [contexts[bs, : 6 - s].reshape(-1, D) for s in range(S)], axis=0
        )
        in_maps.append(
            {
                "ctxT": np.ascontiguousarray(ctx_rows.T.astype(np.float16)),
                "wk5": wk5,
                "wkb": wkb,
                "ench": ench,
                "ident": identm,
                "idx": _build_idx(c, neg_idx),
            }
        )
    return in_maps


def kernel(contexts, encodings, Wk_w, Wk_b, neg_idx, _trace=False):
    in_maps = _prep_in_maps(contexts, encodings, Wk_w, Wk_b, neg_idx)
    nc = build_nc()
    res = run_bass_kernel_spmd(nc, in_maps, list(range(NCORES)), trace=_trace)
    LAST_RUN["exec_time_ns"] = res.exec_time_ns
    LAST_RUN["results"] = res.results
    loss = np.float32(0.0)
    corr = np.float32(0.0)
    for o in res.results:
        loss += np.float32(o["out"][0, 0])
        corr += np.float32(o["out"][0, 1])
    return (
        np.float32(loss / np.float32(N_PREDS)),
        np.float32(corr / np.float32(N_PREDS)),
    )


# revision 27
# speedup vs baseline: 1.1072x; 1.1072x over previous
"""Trainium2 Bass kernel for the CPC loss problem (nn_CPC_85117661872355).

Strategy (data-parallel over batch B across 8 cores):
  - Each core handles 8 of the 64 batch elements: 1120 prediction rows.
  - Phase 1 computes pred TRANSPOSED: predT[e, r] = sum_d Wk[s][e,d] ctx[r,d]
    + b[e], as a single-pass fp16 matmul (Wk blocks stationary, ctx^T rows
    moving, fp32 PSUM accumulate).  E lands on partitions, so the ragged
    per-s row groups go on the free axis and no repack is needed.
  - All 17 logits per row (1 positive + 16 negatives) are dot products
    pred_row . enc_flat[idx], contracted over E.  Target vectors are fetched
    with SWDGE dma_gather(transpose=True) from an fp16 copy of the encoding
    table, which lands them E-on-partitions: gtT[e_sub, e_blk, j].  Each
    128-dot group is then a 10-matmul PE accumulation
    out[r, j] = sum_e predT[e, r] gtT[e, j] whose DIAGONAL holds the dots;
    a fused DVE scalar_tensor_tensor against a host-supplied identity
    extracts diag+accumulates in one [128,128] op.  This moves the 25M
    multiply-adds of the dot products from DVE (1 elem/cycle) to the PE.
  - Gathering the positive through the same path keeps bitwise ties when a
    negative index collides with the positive (jnp.argmax first-index
    tie-break).
  - Dots tiles for all 9 supergroups are kept; softmax-CE runs once at the
    end (one Exp table load, one Ln), accumulating loss/correct per
    partition; a ones-matmul reduces to [1,2] per core; host sums cores.
"""

import functools

import numpy as np

import concourse.bass as bass
import concourse.mybir as mybir
import concourse.tile as tile
from concourse import bacc
from concourse.bass_utils import run_bass_kernel_spmd

F32 = mybir.dt.float32
FP16 = mybir.dt.float16

B, G, D = 64, 7, 1280
S, NEG = 5, 16
NCORES = 8
BSH = B // NCORES  # 8
NS = [BSH * (6 - s) * G for s in range(S)]  # [336, 280, 224, 168, 112]
SOFF = [0]
for n in NS:
    SOFF.append(SOFF[-1] + n)
NR = SOFF[-1]  # 1120 rows per core
NSG = 9  # supergroups of 128 rows
NDOT = 17  # 1 positive + 16 negatives
GCHUNKS = [(0, 6), (6, 6), (12, 5)]  # gather (goff, width) per supergroup
IDX_PER_SG = NDOT * 128  # 2176
IDX_TOT = NSG * IDX_PER_SG  # 19584
N_PREDS = B * G * 20  # 8960

# Results of the last device run (for test harness introspection)
LAST_RUN = {}


@functools.lru_cache(maxsize=1)
def build_nc() -> bass.Bass:
    nc = bacc.Bacc(
        "TRN2",
        target_bir_lowering=False,
        debug=False,
        num_devices=NCORES,
        num_swdge_queues=1,
    )
    # ctxT: [d, r] with d split [128 d_sub, 10 d_blk]
    ctxT = nc.declare_dram_parameter("ctxT", [D, NR], FP16, isOutput=False)
    # wk5: [128 d_in_sub, S, 10 d_out_blk(e), 10 d_in_blk, 128 e_sub]
    # element [di, s, eo, do, ei] = Wk_w[s, eo*128+ei, do*128+di]
    wk5 = nc.declare_dram_parameter("wk5", [128, S, 10, 10, 128], FP16,
                                    isOutput=False)
    wkb = nc.declare_dram_parameter("wkb", [1, S, 10, 128], FP16,
                                    isOutput=False)
    ench = nc.declare_dram_parameter("ench", [B * G * G, D], FP16,
                                     isOutput=False)
    ident = nc.declare_dram_parameter("ident", [128, 128], FP16,
                                      isOutput=False)
    idx = nc.declare_dram_parameter(
        "idx", [128, IDX_TOT // 16], mybir.dt.int16, isOutput=False
    )
    out = nc.declare_dram_parameter("out", [1, 2], F32, isOutput=True)

    Alu = mybir.AluOpType
    Act = mybir.ActivationFunctionType
    Ax = mybir.AxisListType

    # supergroups fully covered after each s finishes phase 1
    SG_AFTER_S = [[0, 1], [2, 3], [4, 5], [6], [7, 8]]
    SG_M = [128] * 8 + [96]  # valid rows per supergroup

    with tile.TileContext(nc) as tc:
        with (
            tc.tile_pool(name="const", bufs=1) as constp,
            tc.tile_pool(name="wk", bufs=2) as wkp,
            tc.tile_pool(name="gath", bufs=3) as gathp,
            tc.tile_pool(name="gath1", bufs=2) as gath1p,
            tc.tile_pool(name="dots", bufs=NSG) as dotsp,
            tc.tile_pool(name="small", bufs=4) as smallp,
            tc.tile_pool(name="acc", bufs=1) as accp,
            tc.tile_pool(name="psA", bufs=2, space="PSUM") as psAp,
            tc.tile_pool(name="psD", bufs=3, space="PSUM") as psDp,
            tc.tile_pool(name="psF", bufs=1, space="PSUM") as psFp,
        ):
            # ---- constants ----
            idx_sb = constp.tile([128, IDX_TOT // 16], mybir.dt.int16,
                                 tag="idx")
            nc.sync.dma_start(idx_sb[:, :], idx[:, :])
            ident_sb = constp.tile([128, 128], FP16, tag="ident")
            nc.sync.dma_start(ident_sb[:, :], ident[:, :])
            ones_sb = constp.tile([128, 1], F32, tag="ones")
            nc.vector.memset(ones_sb[:, :], 1.0)
            ones16 = constp.tile([1, 512], FP16, tag="ones16")
            nc.vector.memset(ones16[:, :], 1.0)
            wkb_sb = constp.tile([1, S, 10, 128], FP16, tag="wkb")
            nc.sync.dma_start(wkb_sb[:, :, :, :], wkb[:, :, :, :])

            # resident fp16 ctx^T: [128 d_sub, 10 d_blk, NR rows]
            ctx_sb = constp.tile([128, 10, NR], FP16, tag="ctx")
            ctx_r = ctxT[:, :].rearrange("(do di) r -> di do r", di=128)
            nc.sync.dma_start(ctx_sb[:, :, 0:560], ctx_r[:, :, 0:560])
            nc.sync.dma_start(ctx_sb[:, :, 560:NR], ctx_r[:, :, 560:NR])

            # predT resident: [128 e_sub, 10 e_blk, NR rows] fp16
            predT = constp.tile([128, 10, NR], FP16, tag="predT")

            # CE stat accumulators
            ss9 = accp.tile([128, NSG], F32, tag="ss9")
            pm9 = accp.tile([128, NSG], F32, tag="pm9")
            lc18 = accp.tile([128, 2 * NSG], F32, tag="lc18")
            scr128 = accp.tile([128, 128], FP16, tag="scr128")

            # ---- gathers (transposed): issued with small lookahead ----
            ench_ap = ench[:, :]
            gt_tiles = {}
            gcursor = [0]

            def emit_gathers(upto):
                while gcursor[0] < min(upto, NSG * len(GCHUNKS)):
                    k = gcursor[0]
                    sg, ci = divmod(k, len(GCHUNKS))
                    goff, w = GCHUNKS[ci]
                    nidx = w * 128
                    pool = gathp if w == 6 else gath1p
                    gt = pool.tile([128, 10, nidx], FP16, tag=f"gt{w}")
                    pos0 = sg * IDX_PER_SG + goff * 128
                    nc.gpsimd.dma_gather(
                        gt[:, :, :],
                        ench_ap,
                        idx_sb[:, pos0 // 16 : (pos0 + nidx) // 16],
                        nidx,
                        nidx,
                        D,
                        transpose=True,
                        queue_num=0,
                    )
                    gt_tiles[(sg, ci)] = gt
                    gcursor[0] += 1

            emit_gathers(3)

            dots_tiles = [
                dotsp.tile([128, NDOT], F32, tag="dots", name=f"dots{i}")
                for i in range(NSG)
            ]
            # rows 96.. of supergroup 8 never get extracts; keep them finite
            nc.vector.memset(dots_tiles[8][96:128, :], 0.0)

            def emit_phase2(sg):
                M = SG_M[sg]
                r0 = sg * 128
                dots_t = dots_tiles[sg]
                for ci, (goff, w) in enumerate(GCHUNKS):
                    gt = gt_tiles.pop((sg, ci))
                    for j in range(w):
                        g = goff + j
                        dps = psDp.tile([128, 512], F32, tag="dps")
                        for dblk in range(10):
                            nc.tensor.matmul(
                                dps[:M, 0:128],
                                lhsT=predT[:, dblk, r0 : r0 + M],
                                rhs=gt[:, dblk, j * 128 : (j + 1) * 128],
                                start=(dblk == 0),
                                stop=(dblk == 9),
                            )
                        # diag extract: accum((psum * 1.0) * I) -> dots col
                        nc.vector.scalar_tensor_tensor(
                            scr128[:M, :],
                            dps[:M, 0:128],
                            1.0,
                            ident_sb[:M, :],
                            op0=Alu.mult,
                            op1=Alu.mult,
                            accum_out=dots_t[:M, g : g + 1],
                        )
                # per-sg CE stats inline (keeps the tail short)
                negm = smallp.tile([128, 1], F32, tag="negm")
                nc.vector.tensor_reduce(
                    negm[:, :], dots_t[:, :], Ax.X, Alu.max, negate=True
                )
                e_t = smallp.tile([128, NDOT], F32, tag="et")
                nc.scalar.activation(
                    e_t[:, :],
                    dots_t[:, :],
                    Act.Exp,
                    bias=negm[:, 0:1],
                    scale=1.0,
                    accum_out=ss9[:, sg : sg + 1],
                )
                nc.vector.tensor_tensor(
                    pm9[:, sg : sg + 1], dots_t[:, 0:1], negm[:, :], Alu.add
                )
                maxneg = smallp.tile([128, 1], F32, tag="maxneg")
                nc.vector.tensor_reduce(
                    maxneg[:, :], dots_t[:, 1:NDOT], Ax.X, Alu.max
                )
                nc.vector.tensor_tensor(
                    lc18[:, NSG + sg : NSG + sg + 1],
                    dots_t[:, 0:1],
                    maxneg[:, :],
                    Alu.is_ge,
                )
                emit_gathers(len(GCHUNKS) * (sg + 1) + 3)

            # ---- phase 1: predT = Wk[s] @ ctx^T + b (single-pass fp16) ----
            for s in range(S):
                n = NS[s]
                o = SOFF[s]
                wk_t = wkp.tile([128, 10, 10, 128], FP16, tag="wk")
                for dch in range(2):
                    nc.sync.dma_start(
                        wk_t[:, 5 * dch : 5 * dch + 5, :, :],
                        wk5[:, s, 5 * dch : 5 * dch + 5, :, :],
                    )
                for eblk in range(10):
                    ps = psAp.tile([128, 512], F32, tag="ps")
                    for dblk in range(10):
                        nc.tensor.matmul(
                            ps[:, 0:n],
                            lhsT=wk_t[:, eblk, dblk, :],
                            rhs=ctx_sb[:, dblk, o : o + n],
                            start=(dblk == 0),
                            stop=False,
                        )
                    # bias: predT[e, r] += b[e] * 1
                    nc.tensor.matmul(
                        ps[:, 0:n],
                        lhsT=wkb_sb[0:1, s, eblk, :],
                        rhs=ones16[0:1, 0:n],
                        start=False,
                        stop=True,
                    )
                    nc.scalar.copy(predT[:, eblk, o : o + n], ps[:, 0:n])
                for sg in SG_AFTER_S[s]:
                    emit_phase2(sg)

            # ---- CE finale: loss_r = ln(ss) - (pos - m) ----
            ln9 = smallp.tile([128, NSG], F32, tag="ln9")
            nc.scalar.activation(ln9[:, :], ss9[:, :], Act.Ln)
            nc.vector.tensor_tensor(
                lc18[:, 0:NSG], ln9[:, :], pm9[:, :], Alu.subtract
            )
            # zero the 32 invalid rows of supergroup 8
            nc.vector.memset(lc18[96:128, 8:9], 0.0)
            nc.vector.memset(lc18[96:128, NSG + 8 : NSG + 9], 0.0)

            # ---- final partition reduce: [128,18] -> [1,18] -> [1,2] ----
            psf = psFp.tile([1, 2 * NSG], F32, tag="psf")
            nc.tensor.matmul(
                psf[:, :], lhsT=ones_sb[:, 0:1], rhs=lc18[:, :], start=True,
                stop=True,
            )
            sum18 = smallp.tile([1, 2 * NSG], F32, tag="sum18")
            nc.vector.tensor_copy(sum18[:, :], psf[:, :])
            outsb = smallp.tile([1, 2], F32, tag="outsb")
            nc.vector.tensor_reduce(
                outsb[:, 0:2],
                sum18[:, :].rearrange("p (a b) -> p a b", a=2),
                Ax.X,
                Alu.add,
            )
            nc.sync.dma_start(out[:, :], outsb[:, :])

    nc.compile()
    return nc


def _row_targets(core: int, neg_idx: np.ndarray) -> np.ndarray:
    """[NR, 17] int array: flat enc index of positive + 16 negatives per row."""
    tg = np.zeros((NR, NDOT), np.int64)
    ri = 0
    for s in range(S):
        rows = 6 - s
        for b in range(BSH):
            bg = core * BSH + b
            for r in range(rows):
                for c7 in range(G):
                    tg[ri, 0] = bg * G * G + (s + 1 + r) * G + c7
                    tg[ri, 1:] = neg_idx[bg, s, r, c7]
                    ri += 1
    assert ri == NR
    return tg


def _build_idx(core: int, neg_idx: np.ndarray) -> np.ndarray:
    """int16 [128, IDX_TOT//16] gather-index tensor in SWDGE wrap layout."""
    tg = _row_targets(core, neg_idx)
    tg_pad = np.zeros((NSG * 128, NDOT), np.int64)
    tg_pad[:NR] = tg
    # list position sg*2176 + g*128 + p  ->  target of (row sg*128+p, dot g)
    lst = tg_pad.reshape(NSG, 128, NDOT).transpose(0, 2, 1).reshape(-1)
    arr = lst.astype(np.int16).reshape(-1, 16).T  # [16, IDX_TOT//16]
    return np.ascontiguousarray(np.tile(arr, (8, 1)))  # [128, ...]


def _prep_in_maps(contexts, encodings, Wk_w, Wk_b, neg_idx):
    contexts = np.ascontiguousarray(np.asarray(contexts, np.float32))
    encodings = np.ascontiguousarray(np.asarray(encodings, np.float32))
    Wk_w = np.ascontiguousarray(np.asarray(Wk_w, np.float32))
    Wk_b = np.ascontiguousarray(np.asarray(Wk_b, np.float32))
    neg_idx = np.asarray(neg_idx)

    ench = np.ascontiguousarray(
        encodings.reshape(B * G * G, D).astype(np.float16)
    )
    # wk5[di, s, eo, do, ei] = Wk_w[s, eo*128+ei, do*128+di]
    wk5 = np.ascontiguousarray(
        Wk_w.reshape(S, 10, 128, 10, 128)
        .transpose(4, 0, 1, 3, 2)
        .astype(np.float16)
    )
    wkb = np.ascontiguousarray(
        Wk_b.reshape(1, S, 10, 128).astype(np.float16)
    )
    identm = np.ascontiguousarray(np.eye(128, dtype=np.float16))

    in_maps = []
    for c in range(NCORES):
        bs = slice(c * BSH, (c + 1) * BSH)
        ctx_rows = np.concatenate(
            [contexts[bs, : 6 - s].reshape(-1, D) for s in range(S)], axis=0
        )
        in_maps.append(
            {
                "ctxT": np.ascontiguousarray(ctx_rows.T.astype(np.float16)),
                "wk5": wk5,
                "wkb": wkb,
                "ench": ench,
                "ident": identm,
                "idx": _build_idx(c, neg_idx),
            }
        )
    return in_maps


def kernel(contexts, encodings, Wk_w, Wk_b, neg_idx, _trace=False):
    in_maps = _prep_in_maps(contexts, encodings, Wk_w, Wk_b, neg_idx)
    nc = build_nc()
    res = run_bass_kernel_spmd(nc, in_maps, list(range(NCORES)), trace=_trace)
    LAST_RUN["exec_time_ns"] = res.exec_time_ns
    LAST_RUN["results"] = res.results
    loss = np.float32(0.0)
    corr = np.float32(0.0)
    for o in res.results:
        loss += np.float32(o["out"][0, 0])
        corr += np.float32(o["out"][0, 1])
    return (
        np.float32(loss / np.float32(N_PREDS)),
        np.float32(corr / np.float32(N_PREDS)),
    )
